# revision 3
# baseline (speedup 1.0000x reference)
"""Trainium2 Bass kernel for nn_AggregationLayer (per-class masked reductions + Hough voting).

Strategy (8 NeuronCores, data-parallel over batch: 2 samples/core):
  Device computes, per (class c in 1..6, sample b), the 20 masked sums
      S_c[x] = sum_p [cat_p == c] * x_p
  for channels x in {1, q0..q3, s0..s2, z, dxh2, dyh2, m, dxh2*pu, m*pv, m*pu,
  dyh2*pv, puA, puB, pvA, pvB}, where (dxh2, dyh2, m) = (dx^2, dy^2, dx*dy) / (|xy|^2 + eps)
  and pu/pv are pixel column/row coordinates (split into two exactly-bf16
  planes puA = 4*(pu//4), puB = pu%4 so position sums are exact).

  The segmented (per-class) reduction runs on the TensorEngine: for every
  128-pixel chunk, one self-loading matmul with stationary = the chunk's
  6 one-hot columns and moving = the chunk's 20 channel values, accumulating
  [6, 20] per-class sums in PSUM across all 2400 chunks of a sample.
  Elementwise channel builds run on DVE/ACT in parallel.

  Host does only the tiny [6, B] finalization: 2x2 solve for the Hough
  center, quaternion -> rotation matrix, intrinsics backprojection, packing.
"""

import numpy as np
import ml_dtypes

B, H, W = 16, 480, 640
CLASSES = 7
C1 = CLASSES - 1
NCORES = 8
SPC = B // NCORES          # samples per core
NPART = 128
COLS = (H * W) // NPART    # 2400
SLAB = 800
NSLAB = COLS // SLAB       # 3
NCH = 20                   # moving channels
DELTA = 1e-12              # guard for 1/(n2 + DELTA)
EPS = 1e-6                 # matches reference

BF16 = ml_dtypes.bfloat16

# moving-channel slot map
S_ONE, S_Q, S_S, S_Z = 0, 1, 5, 8
S_DXH2, S_DYH2, S_M = 9, 10, 11
S_T1, S_T2, S_T3, S_T4 = 12, 13, 14, 15
S_PUA, S_PUB, S_PVA, S_PVB = 16, 17, 18, 19

_NC_CACHE = {}
_STATIC_CACHE = {}


def _build_static():
    if "st" in _STATIC_CACHE:
        return _STATIC_CACHE["st"]
    p = np.arange(H * W, dtype=np.int64)
    pu = (p % W).astype(np.float64)
    pv = (p // W).astype(np.float64)
    puA = (pu // 4) * 4.0
    puB = pu % 4
    pvA = (pv // 4) * 4.0
    pvB = pv % 4
    ones = np.ones_like(pu)

    def plane16(a):
        return a.reshape(NPART, COLS).astype(BF16)

    st16 = np.stack([plane16(ones), plane16(puA), plane16(puB),
                     plane16(pvA), plane16(pvB)])            # [5,128,2400] bf16
    st32 = np.stack([pu.reshape(NPART, COLS).astype(np.float32),
                     pv.reshape(NPART, COLS).astype(np.float32)])  # [2,128,2400] f32
    _STATIC_CACHE["st"] = (st16, st32)
    return st16, st32


def _build_nc():
    if "nc" in _NC_CACHE:
        return _NC_CACHE["nc"]
    import concourse.bacc as bacc
    import concourse.mybir as mybir
    import concourse.tile as tile

    F32, MBF16 = mybir.dt.float32, mybir.dt.bfloat16
    AOT = mybir.AluOpType
    ACTF = mybir.ActivationFunctionType

    nc = bacc.Bacc("TRN2", target_bir_lowering=False, debug=False)
    feat_d = nc.dram_tensor("feat", [SPC, 8, NPART, COLS], MBF16, kind="ExternalInput")
    cat_d = nc.dram_tensor("cat", [SPC, NPART, COLS], MBF16, kind="ExternalInput")
    xy_d = nc.dram_tensor("xy", [SPC, 2, NPART, COLS], MBF16, kind="ExternalInput")
    st16_d = nc.dram_tensor("st16", [5, NPART, COLS], MBF16, kind="ExternalInput")
    st32_d = nc.dram_tensor("st32", [2, NPART, COLS], F32, kind="ExternalInput")
    sums_d = nc.dram_tensor("sums", [C1, SPC * NCH], F32, kind="ExternalOutput")

    with tile.TileContext(nc) as tc:
        with (
            tc.tile_pool(name="mov", bufs=1) as pmov,
            tc.tile_pool(name="stat", bufs=1) as pstat,
            tc.tile_pool(name="work", bufs=2) as pwork,
            tc.tile_pool(name="tmp", bufs=1) as ptmp,
            tc.tile_pool(name="psum", bufs=1, space="PSUM") as pps,
        ):
            # persistent moving buffers, one per slab phase (3-deep rotation)
            m_bufs = []
            for k in range(NSLAB):
                mb = pmov.tile([NPART, NCH * SLAB], MBF16, name=f"Mbuf{k}", tag=f"Mbuf{k}")
                m_bufs.append(mb)
            pu32s, pv32s = [], []
            for k in range(NSLAB):
                pu_t = pstat.tile([NPART, SLAB], F32, name=f"PU{k}", tag=f"PU{k}")
                pv_t = pstat.tile([NPART, SLAB], F32, name=f"PV{k}", tag=f"PV{k}")
                pu32s.append(pu_t)
                pv32s.append(pv_t)

            # one-time static loads: slots 0,16..19 of each M buffer + pu/pv f32
            for k in range(NSLAB):
                sl = slice(k * SLAB, (k + 1) * SLAB)
                mb = m_bufs[k]
                nc.sync.dma_start(mb[:, S_ONE * SLAB:(S_ONE + 1) * SLAB], st16_d.ap()[0, :, sl])
                nc.sync.dma_start(
                    mb[:, S_PUA * SLAB:(S_PVB + 1) * SLAB],
                    st16_d.ap()[1:5].rearrange("a p c -> p a c")[:, :, sl],
                )
                nc.sync.dma_start(pu32s[k][:], st32_d.ap()[0, :, sl])
                nc.sync.dma_start(pv32s[k][:], st32_d.ap()[1, :, sl])

            ps = pps.tile([C1, SPC * NCH], F32)

            delta_t = pstat.tile([NPART, 1], F32, name="delta", tag="delta")
            nc.vector.memset(delta_t[:], DELTA)

            for s in range(SPC):
                for k in range(NSLAB):
                    sl = slice(k * SLAB, (k + 1) * SLAB)
                    mb = m_bufs[k]
                    # --- loads ---
                    nc.sync.dma_start(
                        mb[:, S_Q * SLAB:(S_Z + 1) * SLAB],
                        feat_d.ap()[s].rearrange("a p c -> p a c")[:, :, sl],
                    )
                    cat_t = pwork.tile([NPART, SLAB], MBF16, name=f"cat_{s}_{k}", tag="cat")
                    nc.sync.dma_start(cat_t[:], cat_d.ap()[s, :, sl])
                    x0 = pwork.tile([NPART, SLAB], MBF16, name=f"x0_{s}_{k}", tag="x0")
                    nc.sync.dma_start(x0[:], xy_d.ap()[s, 0, :, sl])
                    x1 = pwork.tile([NPART, SLAB], MBF16, name=f"x1_{s}_{k}", tag="x1")
                    nc.sync.dma_start(x1[:], xy_d.ap()[s, 1, :, sl])

                    # --- one-hot masks (DVE tensor_scalar is_equal, bf16 4x) ---
                    oh = pwork.tile([NPART, C1 * SLAB], MBF16, name=f"oh_{s}_{k}", tag="oh")
                    for c in range(1, CLASSES):
                        nc.vector.tensor_scalar(
                            oh[:, (c - 1) * SLAB:c * SLAB], cat_t[:],
                            float(c), None, op0=AOT.is_equal,
                        )

                    # --- normalization weights ---
                    sx = ptmp.tile([NPART, SLAB], F32, name=f"sx_{s}_{k}", tag="sx")
                    nc.scalar.square(sx[:], x0[:])
                    sy = ptmp.tile([NPART, SLAB], F32, name=f"sy_{s}_{k}", tag="sy")
                    nc.scalar.square(sy[:], x1[:])
                    sxy = ptmp.tile([NPART, SLAB], F32, name=f"sxy_{s}_{k}", tag="sxy")
                    nc.vector.tensor_tensor(sxy[:], x0[:], x1[:], op=AOT.mult)
                    n2 = ptmp.tile([NPART, SLAB], F32, name=f"n2_{s}_{k}", tag="n2")
                    nc.vector.tensor_tensor(n2[:], sx[:], sy[:], op=AOT.add)
                    rr = ptmp.tile([NPART, SLAB], F32, name=f"rr_{s}_{k}", tag="rr")
                    nc.scalar.activation(rr[:], n2[:], ACTF.Abs_reciprocal_sqrt, bias=delta_t[:])
                    r2 = ptmp.tile([NPART, SLAB], F32, name=f"r2_{s}_{k}", tag="r2")
                    nc.scalar.square(r2[:], rr[:])

                    # --- derived channels into M slots ---
                    dxh2 = mb[:, S_DXH2 * SLAB:(S_DXH2 + 1) * SLAB]
                    dyh2 = mb[:, S_DYH2 * SLAB:(S_DYH2 + 1) * SLAB]
                    mm_ = mb[:, S_M * SLAB:(S_M + 1) * SLAB]
                    nc.vector.tensor_tensor(dxh2, sx[:], r2[:], op=AOT.mult)
                    nc.vector.tensor_tensor(dyh2, sy[:], r2[:], op=AOT.mult)
                    nc.vector.tensor_tensor(mm_, sxy[:], r2[:], op=AOT.mult)
                    nc.vector.tensor_tensor(
                        mb[:, S_T1 * SLAB:(S_T1 + 1) * SLAB], dxh2, pu32s[k][:], op=AOT.mult)
                    nc.vector.tensor_tensor(
                        mb[:, S_T2 * SLAB:(S_T2 + 1) * SLAB], mm_, pv32s[k][:], op=AOT.mult)
                    nc.vector.tensor_tensor(
                        mb[:, S_T3 * SLAB:(S_T3 + 1) * SLAB], mm_, pu32s[k][:], op=AOT.mult)
                    nc.vector.tensor_tensor(
                        mb[:, S_T4 * SLAB:(S_T4 + 1) * SLAB], dyh2, pv32s[k][:], op=AOT.mult)

                    # --- PE segmented-sum stream ---
                    oh_r = oh[:].rearrange("p (c s) -> p c s", c=C1)
                    mv_r = mb[:].rearrange("p (c s) -> p c s", c=NCH)
                    for j in range(SLAB):
                        nc.tensor.matmul(
                            ps[:, s * NCH:(s + 1) * NCH],
                            oh_r[:, :, j],
                            mv_r[:, :, j],
                            start=(k == 0 and j == 0),
                            stop=(k == NSLAB - 1 and j == SLAB - 1),
                            skip_group_check=True,
                        )

            outs = ptmp.tile([C1, SPC * NCH], F32)
            nc.vector.tensor_copy(outs[:], ps[:])
            nc.sync.dma_start(sums_d.ap()[:, :], outs[:])

    nc.compile()
    _NC_CACHE["nc"] = nc
    return nc


def _host_prep(inputs):
    """Build per-core input maps."""
    cat = np.asarray(inputs["cat_mask"])
    quat = np.asarray(inputs["quaternion"], dtype=np.float32)
    scales = np.asarray(inputs["scales"], dtype=np.float32)
    xy = np.asarray(inputs["xy"], dtype=np.float32)
    z = np.asarray(inputs["z"], dtype=np.float32)

    st16, st32 = _build_static()

    feat = np.concatenate(
        [quat.reshape(B, 4, H * W), scales.reshape(B, 3, H * W),
         z.reshape(B, 1, H * W)], axis=1,
    ).reshape(B, 8, NPART, COLS).astype(BF16)
    cat16 = cat.reshape(B, NPART, COLS).astype(BF16)
    xy16 = xy.reshape(B, 2, NPART, COLS).astype(BF16)

    in_maps = []
    for i in range(NCORES):
        sl = slice(i * SPC, (i + 1) * SPC)
        in_maps.append({
            "feat": np.ascontiguousarray(feat[sl]),
            "cat": np.ascontiguousarray(cat16[sl]),
            "xy": np.ascontiguousarray(xy16[sl]),
            "st16": st16,
            "st32": st32,
        })
    return in_maps


def _host_finish(sums_all, intrinsics):
    """sums_all: [B, C1, NCH] float64. Returns [C1, B, 26] float32."""
    S = sums_all  # [B, C1, NCH]
    cnt = S[..., S_ONE]
    denom = np.maximum(cnt, 1.0)
    q_agg = S[..., S_Q:S_Q + 4] / denom[..., None]
    s_agg = S[..., S_S:S_S + 3] / denom[..., None]
    z_agg = S[..., S_Z] / denom

    Axx = cnt - S[..., S_DXH2]
    Ayy = cnt - S[..., S_DYH2]
    Axy = -S[..., S_M]
    Spu = S[..., S_PUA] + S[..., S_PUB]
    Spv = S[..., S_PVA] + S[..., S_PVB]
    rx = Spu - S[..., S_T1] - S[..., S_T2]
    ry = Spv - S[..., S_T3] - S[..., S_T4]

    A = np.empty(S.shape[:2] + (2, 2))
    A[..., 0, 0] = Axx + EPS
    A[..., 0, 1] = Axy
    A[..., 1, 0] = Axy
    A[..., 1, 1] = Ayy + EPS
    rhs = np.stack([rx, ry], axis=-1)
    center = np.linalg.solve(A, rhs[..., None])[..., 0]  # [B, C1, 2]

    qn = q_agg / (np.linalg.norm(q_agg, axis=-1, keepdims=True) + 1e-8)
    w, x, y, zz = qn[..., 0], qn[..., 1], qn[..., 2], qn[..., 3]
    R = np.stack([
        1 - 2 * (y * y + zz * zz), 2 * (x * y - w * zz), 2 * (x * zz + w * y),
        2 * (x * y + w * zz), 1 - 2 * (x * x + zz * zz), 2 * (y * zz - w * x),
        2 * (x * zz - w * y), 2 * (y * zz + w * x), 1 - 2 * (x * x + y * y),
    ], axis=-1).reshape(S.shape[:2] + (3, 3))

    zval = np.exp(z_agg)
    Kinv = np.linalg.inv(np.asarray(intrinsics, dtype=np.float64))
    homog = np.concatenate([center, np.ones(S.shape[:2] + (1,))], axis=-1)
    t = zval[..., None] * np.einsum("ij,bcj->bci", Kinv, homog)

    RT = np.zeros(S.shape[:2] + (4, 4))
    RT[..., :3, :3] = R
    RT[..., :3, 3] = t
    RT[..., 3, 3] = 1.0

    out = np.concatenate(
        [q_agg, s_agg, z_agg[..., None], center, RT.reshape(S.shape[:2] + (16,))],
        axis=-1,
    )  # [B, C1, 26]
    return np.transpose(out, (1, 0, 2)).astype(np.float32)


def kernel(**inputs):
    from concourse.bass_utils import run_bass_kernel_spmd

    nc = _build_nc()
    in_maps = _host_prep(inputs)
    res = run_bass_kernel_spmd(nc, in_maps, core_ids=list(range(NCORES)))
    sums_all = np.empty((B, C1, NCH), dtype=np.float64)
    for i in range(NCORES):
        s = res.results[i]["sums"].astype(np.float64)  # [C1, SPC*NCH]
        for j in range(SPC):
            sums_all[i * SPC + j] = s[:, j * NCH:(j + 1) * NCH]
    return _host_finish(sums_all, inputs["intrinsics"])


# revision 11
# speedup vs baseline: 6422.0754x; 6422.0754x over previous
"""Trainium2 Bass kernel for nn_AggregationLayer (per-class masked reductions + Hough voting).

Strategy (8 NeuronCores, data-parallel over batch: 2 samples/core):
  The device computes, per (class c in 1..6, sample b), 20 masked sums
      S_c[x] = sum_p [cat_p == c] * x_p
  over the 307200 pixels of each sample, for channels x in
      {1, q0..q3, s0..s2, z, dxh2, dyh2, m, dxh2*pu, m*pv, m*pu, dyh2*pv,
       puA, puB, pvA, pvB}
  where (dxh2, dyh2, m) = (dx^2, dy^2, dx*dy) / (|xy|^2 + delta) are the
  Hough direction-matrix terms and pu/pv are pixel column/row coordinates.
  pu is split as puA = 4*(pu//4), puB = pu%4 (both exactly representable in
  bf16, likewise pvA/pvB) so the position sums are exact integers in fp32.

  The segmented (per-class) reduction runs on the TensorEngine: for every
  128-pixel chunk (one column of the [128, 2400] plane layout), one
  self-loading bf16 matmul contracts the chunk: stationary = the chunk's 6
  one-hot columns, moving = its 20 channel values (strided access across the
  stacked channel planes), accumulating [6, 20] per-class sums in PSUM across
  all 2400 chunks of a sample. Elementwise channel builds run on DVE/ACT in
  parallel, 800-column slabs triple-buffered against the DMA loads.

  The host does only the tiny [6, B] finalization: 2x2 solve for the Hough
  center, quaternion -> rotation matrix, intrinsics backprojection, packing
  into the [6, 16, 26] output.
"""

import numpy as np
import ml_dtypes

B, H, W = 16, 480, 640
CLASSES = 7
C1 = CLASSES - 1
NCORES = 8
SPC = B // NCORES          # samples per core
NPART = 128
COLS = (H * W) // NPART    # 2400
SLAB = 800
NSLAB = COLS // SLAB       # 3
NCH = 20                   # moving channels
DELTA = 1e-12              # guard for 1/(n2 + DELTA)
EPS = 1e-6                 # matches reference

BF16 = ml_dtypes.bfloat16

# moving-channel slot map
S_ONE, S_Q, S_S, S_Z = 0, 1, 5, 8
S_DXH2, S_DYH2, S_M = 9, 10, 11
S_T1, S_T2, S_T3, S_T4 = 12, 13, 14, 15
S_PUA, S_PUB, S_PVA, S_PVB = 16, 17, 18, 19

_NC_CACHE = {}
_STATIC_CACHE = {}


def _build_static():
    if "st" in _STATIC_CACHE:
        return _STATIC_CACHE["st"]
    p = np.arange(H * W, dtype=np.int64)
    pu = (p % W).astype(np.float64)
    pv = (p // W).astype(np.float64)
    puA = (pu // 4) * 4.0
    puB = pu % 4
    pvA = (pv // 4) * 4.0
    pvB = pv % 4
    ones = np.ones_like(pu)

    def plane16(a):
        return a.reshape(NPART, COLS).astype(BF16)

    st16 = np.stack([plane16(ones), plane16(puA), plane16(puB),
                     plane16(pvA), plane16(pvB)])            # [5,128,2400] bf16
    st32 = np.stack([pu.reshape(NPART, COLS).astype(np.float32),
                     pv.reshape(NPART, COLS).astype(np.float32)])  # [2,128,2400] f32
    _STATIC_CACHE["st"] = (st16, st32)
    return st16, st32


def _build_nc(reps=1, feat_q="sync"):
    """Build + compile the SPMD Bass program. reps > 1 wraps the whole
    pipeline in a hardware For loop (used only for benchmarking)."""
    key = (reps, feat_q)
    if key in _NC_CACHE:
        return _NC_CACHE[key]
    import contextlib
    import concourse.bacc as bacc
    import concourse.mybir as mybir
    import concourse.tile as tile

    F32, MBF16 = mybir.dt.float32, mybir.dt.bfloat16
    AOT = mybir.AluOpType
    ACTF = mybir.ActivationFunctionType

    nc = bacc.Bacc("TRN2", target_bir_lowering=False, debug=False)
    feat_d = nc.dram_tensor("feat", [SPC, 8, NPART, COLS], MBF16, kind="ExternalInput")
    cat_d = nc.dram_tensor("cat", [SPC, NPART, COLS], MBF16, kind="ExternalInput")
    xy_d = nc.dram_tensor("xy", [SPC, 2, NPART, COLS], MBF16, kind="ExternalInput")
    st16_d = nc.dram_tensor("st16", [5, NPART, COLS], MBF16, kind="ExternalInput")
    st32_d = nc.dram_tensor("st32", [2, NPART, COLS], F32, kind="ExternalInput")
    sums_d = nc.dram_tensor("sums", [C1, SPC * NCH], F32, kind="ExternalOutput")

    with tile.TileContext(nc) as tc:
        with (
            tc.tile_pool(name="mov", bufs=1) as pmov,
            tc.tile_pool(name="stat", bufs=1) as pstat,
            tc.tile_pool(name="work", bufs=2) as pwork,
            tc.tile_pool(name="tmp", bufs=2) as ptmp,
            tc.tile_pool(name="psum", bufs=1, space="PSUM") as pps,
        ):
            # persistent moving buffers, one per slab phase (3-deep rotation);
            # static channel slots (ones/puA/puB/pvA/pvB) are written once per
            # physical buffer and survive the per-sample rewrites of slots 1-15
            m_bufs = []
            for k in range(NSLAB):
                mb = pmov.tile([NPART, NCH * SLAB], MBF16, name=f"Mbuf{k}", tag=f"Mbuf{k}")
                m_bufs.append(mb)
            pu32s, pv32s = [], []
            for k in range(NSLAB):
                pu_t = pstat.tile([NPART, SLAB], F32, name=f"PU{k}", tag=f"PU{k}")
                pv_t = pstat.tile([NPART, SLAB], F32, name=f"PV{k}", tag=f"PV{k}")
                pu32s.append(pu_t)
                pv32s.append(pv_t)

            for k in range(NSLAB):
                sl = slice(k * SLAB, (k + 1) * SLAB)
                mb = m_bufs[k]
                nc.sync.dma_start(mb[:, S_ONE * SLAB:(S_ONE + 1) * SLAB], st16_d.ap()[0, :, sl])
                nc.sync.dma_start(
                    mb[:, S_PUA * SLAB:(S_PVB + 1) * SLAB],
                    st16_d.ap()[1:5].rearrange("a p c -> p a c")[:, :, sl],
                )
                nc.sync.dma_start(pu32s[k][:], st32_d.ap()[0, :, sl])
                nc.sync.dma_start(pv32s[k][:], st32_d.ap()[1, :, sl])

            ps = pps.tile([C1, SPC * NCH], F32)
            delta_t = pstat.tile([NPART, 1], F32, name="delta", tag="delta")
            nc.vector.memset(delta_t[:], DELTA)

            loop_cm = tc.For_i(0, reps, 1) if reps > 1 else contextlib.nullcontext()
            with loop_cm:
              for s in range(SPC):
                for k in range(NSLAB):
                    sl = slice(k * SLAB, (k + 1) * SLAB)
                    mb = m_bufs[k]
                    # --- loads (one contiguous-run DMA per plane) ---
                    feat_eng = nc.gpsimd if feat_q == "gpsimd" else nc.sync
                    for a in range(8):
                        feat_eng.dma_start(
                            mb[:, (S_Q + a) * SLAB:(S_Q + a + 1) * SLAB],
                            feat_d.ap()[s, a, :, sl],
                        )
                    cat_t = pwork.tile([NPART, SLAB], MBF16, name=f"cat_{s}_{k}", tag="cat")
                    x0 = pwork.tile([NPART, SLAB], MBF16, name=f"x0_{s}_{k}", tag="x0")
                    x1 = pwork.tile([NPART, SLAB], MBF16, name=f"x1_{s}_{k}", tag="x1")
                    nc.sync.dma_start(cat_t[:], cat_d.ap()[s, :, sl])
                    nc.sync.dma_start(x0[:], xy_d.ap()[s, 0, :, sl])
                    nc.sync.dma_start(x1[:], xy_d.ap()[s, 1, :, sl])

                    # --- one-hot masks (DVE tensor_scalar is_equal, bf16) ---
                    oh = pwork.tile([NPART, C1 * SLAB], MBF16, name=f"oh_{s}_{k}", tag="oh")
                    for c in range(1, CLASSES):
                        nc.vector.tensor_scalar(
                            oh[:, (c - 1) * SLAB:c * SLAB], cat_t[:],
                            float(c), None, op0=AOT.is_equal,
                        )

                    # --- per-pixel direction weights: r2 = 1/(dx^2+dy^2+delta) ---
                    sx = ptmp.tile([NPART, SLAB], F32, name=f"sx_{s}_{k}", tag="sx")
                    sy = ptmp.tile([NPART, SLAB], F32, name=f"sy_{s}_{k}", tag="sy")
                    nc.scalar.square(sx[:], x0[:])
                    nc.scalar.square(sy[:], x1[:])
                    sxy = ptmp.tile([NPART, SLAB], F32, name=f"sxy_{s}_{k}", tag="sxy")
                    n2 = ptmp.tile([NPART, SLAB], F32, name=f"n2_{s}_{k}", tag="n2")
                    nc.vector.tensor_tensor(sxy[:], x0[:], x1[:], op=AOT.mult)
                    nc.vector.tensor_tensor(n2[:], sx[:], sy[:], op=AOT.add)
                    rr = ptmp.tile([NPART, SLAB], F32, name=f"rr_{s}_{k}", tag="rr")
                    r2 = ptmp.tile([NPART, SLAB], F32, name=f"r2_{s}_{k}", tag="r2")
                    nc.scalar.activation(rr[:], n2[:], ACTF.Abs_reciprocal_sqrt, bias=delta_t[:])
                    nc.scalar.square(r2[:], rr[:])

                    # --- derived channels into M slots ---
                    dxh2 = mb[:, S_DXH2 * SLAB:(S_DXH2 + 1) * SLAB]
                    dyh2 = mb[:, S_DYH2 * SLAB:(S_DYH2 + 1) * SLAB]
                    mm_ = mb[:, S_M * SLAB:(S_M + 1) * SLAB]
                    nc.vector.tensor_tensor(dxh2, sx[:], r2[:], op=AOT.mult)
                    nc.vector.tensor_tensor(dyh2, sy[:], r2[:], op=AOT.mult)
                    nc.vector.tensor_tensor(mm_, sxy[:], r2[:], op=AOT.mult)
                    nc.vector.tensor_tensor(
                        mb[:, S_T1 * SLAB:(S_T1 + 1) * SLAB], dxh2, pu32s[k][:], op=AOT.mult)
                    nc.vector.tensor_tensor(
                        mb[:, S_T2 * SLAB:(S_T2 + 1) * SLAB], mm_, pv32s[k][:], op=AOT.mult)
                    nc.vector.tensor_tensor(
                        mb[:, S_T3 * SLAB:(S_T3 + 1) * SLAB], mm_, pu32s[k][:], op=AOT.mult)
                    nc.vector.tensor_tensor(
                        mb[:, S_T4 * SLAB:(S_T4 + 1) * SLAB], dyh2, pv32s[k][:], op=AOT.mult)

                    # --- PE segmented-sum stream: one matmul per 128-px chunk ---
                    oh_r = oh[:].rearrange("p (c s) -> p c s", c=C1)
                    mv_r = mb[:].rearrange("p (c s) -> p c s", c=NCH)
                    for j in range(SLAB):
                        nc.tensor.matmul(
                            ps[:, s * NCH:(s + 1) * NCH],
                            oh_r[:, :, j],
                            mv_r[:, :, j],
                            start=(k == 0 and j == 0),
                            stop=(k == NSLAB - 1 and j == SLAB - 1),
                            skip_group_check=True,
                        )

            outs = ptmp.tile([C1, SPC * NCH], F32)
            nc.vector.tensor_copy(outs[:], ps[:])
            nc.sync.dma_start(sums_d.ap()[:, :], outs[:])

    nc.compile()
    _NC_CACHE[key] = nc
    return nc


def _host_prep(inputs):
    """Build per-core input maps (bf16 planes in [128, 2400] partition-major layout)."""
    cat = np.asarray(inputs["cat_mask"])
    quat = np.asarray(inputs["quaternion"], dtype=np.float32)
    scales = np.asarray(inputs["scales"], dtype=np.float32)
    xy = np.asarray(inputs["xy"], dtype=np.float32)
    z = np.asarray(inputs["z"], dtype=np.float32)

    st16, st32 = _build_static()

    feat = np.concatenate(
        [quat.reshape(B, 4, H * W), scales.reshape(B, 3, H * W),
         z.reshape(B, 1, H * W)], axis=1,
    ).reshape(B, 8, NPART, COLS).astype(BF16)
    cat16 = cat.reshape(B, NPART, COLS).astype(BF16)
    xy16 = xy.reshape(B, 2, NPART, COLS).astype(BF16)

    in_maps = []
    for i in range(NCORES):
        sl = slice(i * SPC, (i + 1) * SPC)
        in_maps.append({
            "feat": np.ascontiguousarray(feat[sl]),
            "cat": np.ascontiguousarray(cat16[sl]),
            "xy": np.ascontiguousarray(xy16[sl]),
            "st16": st16,
            "st32": st32,
        })
    return in_maps


def _host_finish(sums_all, intrinsics):
    """sums_all: [B, C1, NCH] float64. Returns [C1, B, 26] float32."""
    S = sums_all
    cnt = S[..., S_ONE]
    denom = np.maximum(cnt, 1.0)
    q_agg = S[..., S_Q:S_Q + 4] / denom[..., None]
    s_agg = S[..., S_S:S_S + 3] / denom[..., None]
    z_agg = S[..., S_Z] / denom

    Axx = cnt - S[..., S_DXH2]
    Ayy = cnt - S[..., S_DYH2]
    Axy = -S[..., S_M]
    Spu = S[..., S_PUA] + S[..., S_PUB]
    Spv = S[..., S_PVA] + S[..., S_PVB]
    rx = Spu - S[..., S_T1] - S[..., S_T2]
    ry = Spv - S[..., S_T3] - S[..., S_T4]

    A = np.empty(S.shape[:2] + (2, 2))
    A[..., 0, 0] = Axx + EPS
    A[..., 0, 1] = Axy
    A[..., 1, 0] = Axy
    A[..., 1, 1] = Ayy + EPS
    rhs = np.stack([rx, ry], axis=-1)
    center = np.linalg.solve(A, rhs[..., None])[..., 0]  # [B, C1, 2]

    qn = q_agg / (np.linalg.norm(q_agg, axis=-1, keepdims=True) + 1e-8)
    w, x, y, zz = qn[..., 0], qn[..., 1], qn[..., 2], qn[..., 3]
    R = np.stack([
        1 - 2 * (y * y + zz * zz), 2 * (x * y - w * zz), 2 * (x * zz + w * y),
        2 * (x * y + w * zz), 1 - 2 * (x * x + zz * zz), 2 * (y * zz - w * x),
        2 * (x * zz - w * y), 2 * (y * zz + w * x), 1 - 2 * (x * x + y * y),
    ], axis=-1).reshape(S.shape[:2] + (3, 3))

    zval = np.exp(z_agg)
    Kinv = np.linalg.inv(np.asarray(intrinsics, dtype=np.float64))
    homog = np.concatenate([center, np.ones(S.shape[:2] + (1,))], axis=-1)
    t = zval[..., None] * np.einsum("ij,bcj->bci", Kinv, homog)

    RT = np.zeros(S.shape[:2] + (4, 4))
    RT[..., :3, :3] = R
    RT[..., :3, 3] = t
    RT[..., 3, 3] = 1.0

    out = np.concatenate(
        [q_agg, s_agg, z_agg[..., None], center, RT.reshape(S.shape[:2] + (16,))],
        axis=-1,
    )  # [B, C1, 26]
    return np.transpose(out, (1, 0, 2)).astype(np.float32)


def kernel(**inputs):
    from concourse.bass_utils import run_bass_kernel_spmd

    nc = _build_nc()
    in_maps = _host_prep(inputs)
    res = run_bass_kernel_spmd(nc, in_maps, core_ids=list(range(NCORES)))
    sums_all = np.empty((B, C1, NCH), dtype=np.float64)
    for i in range(NCORES):
        s = res.results[i]["sums"].astype(np.float64)  # [C1, SPC*NCH]
        for j in range(SPC):
            sums_all[i * SPC + j] = s[:, j * NCH:(j + 1) * NCH]
    return _host_finish(sums_all, inputs["intrinsics"])
